# revision 12
# baseline (speedup 1.0000x reference)
"""MultiHeadAttention TRN2 Bass kernel.

Problem: B=2, S=2048, E=1024, H=16, D=64.  8 NeuronCores.
Sharding: core c -> batch b=c//4, head-group g=c%4 (heads 4g..4g+3).
Each core computes, for its (b, g):
  - Q/K/V projections for its 4 heads (column shard of Wq/Wk/Wv)
  - per-head masked softmax attention
  - partial output  out_p = concat_h(attn_h @ V_h) @ Wo_cols.T   [S, E]
  - partial avg     avg_p = (1/16) * sum_h attn_h                [S, S]
Host sums the 4 partials per batch and adds bo.

On-core dataflow (per q-group of 512 columns):
  scoresT [k,q] psum (float32r matmuls from QT/KT head slices)
  -> ACT exp -> fp16 sbuf -> DVE multiply by transposed mask
  -> PE attn @ [V_h | ones] (T layout, no transposes needed): outT psum
     row 64 = softmax denominators -> DVE reciprocal -> PE broadcast
  -> outT normalized at eviction (DVE), attn normalized in place (DVE)
  -> avg: PE transpose-back of attn tiles with identity*(1/16),
     accumulating head pairs in fp16 PSUM; pair sums added at eviction.
"""

import sys

sys.path.insert(0, "/opt/trn_rl_repo")

import numpy as np

B, S, E, H = 2, 2048, 1024, 16
D = E // H  # 64
G = 4  # heads per core
DG = G * D  # 256 dims per core
NCORES = 8
SCALE = float(np.sqrt(D))

_cached = {}


def _build_nc():
    import concourse.bass as bass  # noqa: F401
    import concourse.tile as tile
    from concourse import bacc, mybir

    dt = mybir.dt
    AF = mybir.ActivationFunctionType
    OP = mybir.AluOpType

    P = 128
    EC = E // P  # 8 e-chunks
    DGC = DG // P  # 2 dg-chunks
    KC = S // P  # 16 k-chunks
    NQG = 4  # q groups of 512
    QW = S // NQG  # 512
    SH = 1024  # s-half width for projection streaming
    SLOT = D + 1  # 65
    VW = G * SLOT  # 260

    nc = bacc.Bacc(None, target_bir_lowering=False)

    # ---- DRAM I/O ----
    xq_d = nc.dram_tensor("xq_t", [E, S], dt.float32r, kind="ExternalInput")
    xk_d = nc.dram_tensor("xk_t", [E, S], dt.float32r, kind="ExternalInput")
    xv_d = nc.dram_tensor("xv_t", [E, S], dt.float32r, kind="ExternalInput")
    mask_d = nc.dram_tensor("mask_t", [S, S], dt.float16, kind="ExternalInput")
    wq_d = nc.dram_tensor("wq_t", [E, DG], dt.float32r, kind="ExternalInput")
    wk_d = nc.dram_tensor("wk_t", [E, DG], dt.float32r, kind="ExternalInput")
    wv_d = nc.dram_tensor("wv_t", [E, VW], dt.float32r, kind="ExternalInput")
    wo_d = nc.dram_tensor("wo_t", [DG, E], dt.float16, kind="ExternalInput")
    bqs_d = nc.dram_tensor("bqs", [P, DGC], dt.float32, kind="ExternalInput")
    bks_d = nc.dram_tensor("bks", [P, DGC], dt.float32, kind="ExternalInput")
    bvs_d = nc.dram_tensor("bvs", [1, VW], dt.float32r, kind="ExternalInput")
    ones_d = nc.dram_tensor("ones_c", [1, P], dt.float32, kind="ExternalInput")
    onesr_d = nc.dram_tensor("ones_r", [1, P], dt.float32r, kind="ExternalInput")
    id16_d = nc.dram_tensor("id16", [P, P], dt.float16, kind="ExternalInput")
    out_d = nc.dram_tensor("out_p", [S, E], dt.float32, kind="ExternalOutput")
    avg_d = nc.dram_tensor("avg_p", [S, S], dt.float16, kind="ExternalOutput")

    with tile.TileContext(nc) as tc:
        with (
            tc.tile_pool(name="consts", bufs=1) as consts,
            tc.tile_pool(name="xpool", bufs=8) as xpool,
            tc.tile_pool(name="wpool", bufs=2) as wpool,
            tc.tile_pool(name="qkv", bufs=1) as qkv,
            tc.tile_pool(name="maskp", bufs=3) as maskp,
            tc.tile_pool(name="expp", bufs=3) as expp,
            tc.tile_pool(name="small", bufs=2) as small,
            tc.tile_pool(name="avghalf", bufs=4) as avghalf,
            tc.tile_pool(name="stage", bufs=2) as stage,
            tc.tile_pool(name="ps_sc", bufs=2, space="PSUM") as ps_sc,
            tc.tile_pool(name="ps_ot", bufs=1, space="PSUM") as ps_ot,
            tc.tile_pool(name="ps_av", bufs=1, space="PSUM") as ps_av,
            tc.tile_pool(name="ps_bc", bufs=1, space="PSUM") as ps_bc,
        ):
            # ---- constants ----
            ones_s = consts.tile([1, P], dt.float32)
            nc.sync.dma_start(out=ones_s[:], in_=ones_d[:])
            onesr_s = consts.tile([1, P], dt.float32r)
            nc.sync.dma_start(out=onesr_s[:], in_=onesr_d[:])
            id16_s = consts.tile([P, P], dt.float16)
            nc.sync.dma_start(out=id16_s[:], in_=id16_d[:])
            bqs_s = consts.tile([P, DGC], dt.float32)
            nc.sync.dma_start(out=bqs_s[:], in_=bqs_d[:])
            bks_s = consts.tile([P, DGC], dt.float32)
            nc.sync.dma_start(out=bks_s[:], in_=bks_d[:])
            bvs_s = consts.tile([1, VW], dt.float32r)
            nc.sync.dma_start(out=bvs_s[:], in_=bvs_d[:])

            qt_s = qkv.tile([P, DGC, S], dt.float32r, tag="qt")
            kt_s = qkv.tile([P, DGC, S], dt.float32r, tag="kt")
            v_s = qkv.tile([P, KC, VW], dt.float16, tag="v")
            outn_s = qkv.tile([P, DGC, S], dt.float16, tag="outn")

            # ---- Q/K projections: psum[dg,qw] = sum_ec wT_ec.T @ x_ec ----
            for x_d, wname, wdram, b_s, out_s, scal in (
                (xq_d, "wq", wq_d, bqs_s, qt_s, 1.0 / SCALE),
                (xk_d, "wk", wk_d, bks_s, kt_s, 1.0),
            ):
                w_s = wpool.tile([P, EC, DG], dt.float32r, tag="w")
                nc.sync.dma_start(
                    out=w_s[:], in_=wdram[:].rearrange("(ec p) d -> p ec d", p=P)
                )
                for sh in range(S // SH):
                    xch = []
                    for ec in range(EC):
                        xc = xpool.tile([P, SH], dt.float32r, tag="xc")
                        nc.sync.dma_start(
                            out=xc[:],
                            in_=x_d[ec * P : (ec + 1) * P, sh * SH : (sh + 1) * SH],
                        )
                        xch.append(xc)
                    for dgc in range(DGC):
                        for sb in range(SH // QW):
                            pp = ps_sc.tile([P, QW], dt.float32, tag="sc")
                            for ec in range(EC):
                                nc.tensor.matmul(
                                    pp[:],
                                    w_s[:, ec, dgc * P : (dgc + 1) * P],
                                    xch[ec][:, sb * QW : (sb + 1) * QW],
                                    start=(ec == 0),
                                    stop=(ec == EC - 1),
                                )
                            nc.scalar.activation(
                                out_s[
                                    :,
                                    dgc,
                                    sh * SH + sb * QW : sh * SH + (sb + 1) * QW,
                                ],
                                pp[:],
                                AF.Identity,
                                bias=b_s[:, dgc : dgc + 1],
                                scale=scal,
                            )
            # ---- V projection: psum[s,VW] = bias + sum_ec x_ec.T @ wvT_ec ----
            wv_s = wpool.tile([P, EC, VW], dt.float32r, tag="w")
            nc.sync.dma_start(
                out=wv_s[:], in_=wv_d[:].rearrange("(ec p) d -> p ec d", p=P)
            )
            for sh in range(S // SH):
                xch = []
                for ec in range(EC):
                    xc = xpool.tile([P, SH], dt.float32r, tag="xc")
                    nc.sync.dma_start(
                        out=xc[:],
                        in_=xv_d[ec * P : (ec + 1) * P, sh * SH : (sh + 1) * SH],
                    )
                    xch.append(xc)
                for sb in range(SH // P):
                    pv = ps_sc.tile([P, VW], dt.float32, tag="sc")
                    nc.tensor.matmul(
                        pv[:], onesr_s[:], bvs_s[:], start=True, stop=False
                    )
                    for ec in range(EC):
                        nc.tensor.matmul(
                            pv[:],
                            xch[ec][:, sb * P : (sb + 1) * P],
                            wv_s[:, ec, :],
                            start=False,
                            stop=(ec == EC - 1),
                        )
                    nc.scalar.copy(v_s[:, sh * (SH // P) + sb, :], pv[:])

            # ---- attention ----
            ah_list = [None] * (QW // P)
            for qg in range(NQG):
                mk_halves = []
                for mh in range(2):
                    mk_s = maskp.tile([P, KC, QW // 2], dt.float16, tag="mask")
                    q0 = qg * QW + mh * (QW // 2)
                    nc.sync.dma_start(
                        out=mk_s[:],
                        in_=mask_d[:].rearrange("(kc p) q -> p kc q", p=P)[
                            :, :, q0 : q0 + QW // 2
                        ],
                    )
                    mk_halves.append(mk_s)
                for pair in range(2):
                    pair_exp = []
                    for hh in range(2):
                        h = pair * 2 + hh
                        dgc = h // 2
                        prow = (h % 2) * D
                        ex_s = expp.tile([P, KC, QW], dt.float16, tag="expt")
                        pair_exp.append(ex_s)
                        # scoresT [k, q] in kc pairs; exp psum->sbuf fp16
                        for kp in range(KC // 2):
                            sc = ps_sc.tile([P, 2 * QW], dt.float32, tag="sc")
                            for j in range(2):
                                kc = 2 * kp + j
                                nc.tensor.matmul(
                                    sc[:, j * QW : (j + 1) * QW],
                                    kt_s[
                                        prow : prow + D, dgc, kc * P : (kc + 1) * P
                                    ],
                                    qt_s[
                                        prow : prow + D,
                                        dgc,
                                        qg * QW : (qg + 1) * QW,
                                    ],
                                    start=True,
                                    stop=True,
                                )
                            nc.scalar.activation(
                                ex_s[:, 2 * kp : 2 * kp + 2, :].rearrange(
                                    "p a b -> p (a b)"
                                ),
                                sc[:],
                                AF.Exp,
                            )
                        # mask multiply (in place, fp16)
                        for mh in range(2):
                            qs = mh * (QW // 2)
                            nc.vector.scalar_tensor_tensor(
                                out=ex_s[:, :, qs : qs + QW // 2],
                                in0=ex_s[:, :, qs : qs + QW // 2],
                                scalar=1.0,
                                in1=mk_halves[mh][:],
                                op0=OP.mult,
                                op1=OP.mult,
                            )
                        # attn @ [V|1] : outT psum [65, qw]
                        ot = ps_ot.tile([SLOT, QW], dt.float32, tag="ot")
                        for kc in range(KC):
                            nc.tensor.matmul(
                                ot[:],
                                v_s[:, kc, h * SLOT : (h + 1) * SLOT],
                                ex_s[:, kc, :],
                                start=(kc == 0),
                                stop=(kc == KC - 1),
                            )
                        # reciprocal of sums; broadcast to 128 partitions
                        rrow = small.tile([1, QW], dt.float32, tag="rrow")
                        nc.vector.reciprocal(rrow[:], ot[D : D + 1, :])
                        bc = ps_bc.tile([P, QW], dt.float32, tag="bc")
                        nc.tensor.matmul(
                            bc[:], ones_s[:], rrow[:], start=True, stop=True
                        )
                        bc16 = small.tile([P, QW], dt.float16, tag="bc16")
                        nc.vector.tensor_copy(bc16[:], bc[:])
                        bc32 = small.tile([D, QW], dt.float32, tag="bc32")
                        nc.scalar.copy(bc32[:], bc[:D, :])
                        # normalized head output -> outn (fp16)
                        nc.vector.tensor_mul(
                            outn_s[
                                prow : prow + D, dgc, qg * QW : (qg + 1) * QW
                            ],
                            ot[:D, :],
                            bc32[:],
                        )
                        # normalize attn in place for the avg path
                        nc.vector.scalar_tensor_tensor(
                            out=ex_s[:],
                            in0=ex_s[:],
                            scalar=1.0,
                            in1=bc16[:].unsqueeze(1).broadcast_to([P, KC, QW]),
                            op0=OP.mult,
                            op1=OP.mult,
                        )
                    # avg: transpose-back this head pair via regular fp16
                    # matmuls against the identity (transpose-mode is a pure
                    # permutation: rhs values ignored, no accumulation).
                    # fp32 psum accumulator, k-halves; 1/16 applied at evict.
                    for sq in range(QW // P):
                        avs = None
                        if pair == 1:
                            avs = stage.tile([P, S], dt.float16, tag="stage")
                        for kh in range(2):
                            av = ps_av.tile([P, S // 2], dt.float32, tag="av")
                            for kk in range(KC // 2):
                                kc = kh * (KC // 2) + kk
                                for hh in range(2):
                                    nc.tensor.matmul(
                                        av[:, kk * P : (kk + 1) * P],
                                        pair_exp[hh][:, kc, sq * P : (sq + 1) * P],
                                        id16_s[:],
                                        start=(hh == 0),
                                        stop=(hh == 1),
                                    )
                            ks = slice(kh * (S // 2), (kh + 1) * (S // 2))
                            if pair == 0:
                                if kh == 0:
                                    ah_list[sq] = avghalf.tile(
                                        [P, S], dt.float16, tag="ah", name="ah"
                                    )
                                nc.vector.tensor_scalar_mul(
                                    ah_list[sq][:, ks], av[:], 1.0 / 16.0
                                )
                            else:
                                nc.vector.scalar_tensor_tensor(
                                    out=avs[:, ks],
                                    in0=av[:],
                                    scalar=1.0 / 16.0,
                                    in1=ah_list[sq][:, ks],
                                    op0=OP.mult,
                                    op1=OP.add,
                                )
                        if pair == 1:
                            r0 = qg * QW + sq * P
                            nc.sync.dma_start(
                                out=avg_d[r0 : r0 + P, :], in_=avs[:]
                            )

            # ---- output projection ----
            wo_s = wpool.tile([P, DGC, E], dt.float16, tag="w")
            nc.sync.dma_start(
                out=wo_s[:], in_=wo_d[:].rearrange("(dc p) e -> p dc e", p=P)
            )
            for sb in range(KC):
                po = ps_sc.tile([P, E], dt.float32, tag="sc")
                for eb in range(2):
                    for dgc in range(DGC):
                        nc.tensor.matmul(
                            po[:, eb * QW : (eb + 1) * QW],
                            outn_s[:, dgc, sb * P : (sb + 1) * P],
                            wo_s[:, dgc, eb * QW : (eb + 1) * QW],
                            start=(dgc == 0),
                            stop=(dgc == DGC - 1),
                        )
                os_ = stage.tile([P, E], dt.float32, tag="stage")
                nc.scalar.copy(os_[:], po[:])
                nc.sync.dma_start(
                    out=out_d[sb * P : (sb + 1) * P, :], in_=os_[:]
                )

    nc.finalize()
    return nc


def _prep_core_inputs(inputs, b, g):
    f32 = np.float32
    q = np.ascontiguousarray(inputs["query"][b].T).astype(f32, copy=False)
    k = np.ascontiguousarray(inputs["key"][b].T).astype(f32, copy=False)
    v = np.ascontiguousarray(inputs["value"][b].T).astype(f32, copy=False)
    mask_t = np.ascontiguousarray(inputs["mask"][b].T).astype(np.float16)
    r0, r1 = g * DG, (g + 1) * DG
    wq_t = np.ascontiguousarray(inputs["Wq"][r0:r1, :].T).astype(f32, copy=False)
    wk_t = np.ascontiguousarray(inputs["Wk"][r0:r1, :].T).astype(f32, copy=False)
    wv = inputs["Wv"][r0:r1, :]  # [DG, E]
    SLOT = D + 1
    wv_t = np.zeros((E, G * SLOT), dtype=f32)
    bvs = np.zeros((1, G * SLOT), dtype=f32)
    bv = inputs["bv"][r0:r1]
    for h in range(G):
        wv_t[:, h * SLOT : h * SLOT + D] = wv[h * D : (h + 1) * D, :].T
        bvs[0, h * SLOT : h * SLOT + D] = bv[h * D : (h + 1) * D]
        bvs[0, h * SLOT + D] = 1.0
    wo_t = np.ascontiguousarray(inputs["Wo"][:, r0:r1].T).astype(np.float16)
    bqs = np.ascontiguousarray(inputs["bq"][r0:r1].reshape(2, 128).T).astype(f32)
    bks = np.ascontiguousarray(inputs["bk"][r0:r1].reshape(2, 128).T).astype(f32)
    ones_c = np.ones((1, 128), dtype=f32)
    id16 = np.eye(128, dtype=np.float16)
    return {
        "xq_t": q,
        "xk_t": k,
        "xv_t": v,
        "mask_t": mask_t,
        "wq_t": wq_t,
        "wk_t": wk_t,
        "wv_t": wv_t,
        "wo_t": wo_t,
        "bqs": bqs,
        "bks": bks,
        "bvs": bvs,
        "ones_c": ones_c,
        "ones_r": ones_c.copy(),
        "id16": id16,
    }


def kernel(query, key, value, mask, Wq, bq, Wk, bk, Wv, bv, Wo, bo, trace=False):
    from concourse.bass_utils import run_bass_kernel_spmd

    inputs = {
        "query": np.asarray(query),
        "key": np.asarray(key),
        "value": np.asarray(value),
        "mask": np.asarray(mask),
        "Wq": np.asarray(Wq),
        "bq": np.asarray(bq),
        "Wk": np.asarray(Wk),
        "bk": np.asarray(bk),
        "Wv": np.asarray(Wv),
        "bv": np.asarray(bv),
        "Wo": np.asarray(Wo),
        "bo": np.asarray(bo),
    }
    if "nc" not in _cached:
        _cached["nc"] = _build_nc()
    nc = _cached["nc"]

    in_maps = [_prep_core_inputs(inputs, c // G, c % G) for c in range(NCORES)]

    res = run_bass_kernel_spmd(nc, in_maps, core_ids=list(range(NCORES)), trace=trace)
    _cached["last_result"] = res

    output = np.zeros((B, S, E), dtype=np.float32)
    avg = np.zeros((B, S, S), dtype=np.float32)
    for c in range(NCORES):
        b = c // G
        output[b] += res.results[c]["out_p"]
        avg[b] += res.results[c]["avg_p"].astype(np.float32)
    output += inputs["bo"][None, None, :]
    return output, avg


if __name__ == "__main__":
    rng = np.random.default_rng(0)
    inp = {
        "query": rng.standard_normal((B, S, E), dtype=np.float32),
        "key": rng.standard_normal((B, S, E), dtype=np.float32),
        "value": rng.standard_normal((B, S, E), dtype=np.float32),
        "mask": rng.integers(0, 2, (B, S, S)).astype(np.int32),
        "Wq": (rng.standard_normal((E, E), dtype=np.float32) / np.sqrt(E)),
        "bq": np.zeros(E, np.float32),
        "Wk": (rng.standard_normal((E, E), dtype=np.float32) / np.sqrt(E)),
        "bk": np.zeros(E, np.float32),
        "Wv": (rng.standard_normal((E, E), dtype=np.float32) / np.sqrt(E)),
        "bv": np.zeros(E, np.float32),
        "Wo": (rng.standard_normal((E, E), dtype=np.float32) / np.sqrt(E)),
        "bo": np.zeros(E, np.float32),
    }
    out, avg = kernel(**inp)
    print("out", out.shape, "avg", avg.shape)


# revision 24
# speedup vs baseline: 1.1039x; 1.1039x over previous
"""MultiHeadAttention TRN2 Bass kernel.

Problem: B=2, S=2048, E=1024, H=16, D=64.  8 NeuronCores.
Sharding: core c -> batch b=c//4, head-group g=c%4 (heads 4g..4g+3).
Each core computes, for its (b, g):
  - Q/K/V projections for its 4 heads (column shard of Wq/Wk/Wv)
  - per-head masked softmax attention
  - partial output  out_p = concat_h(attn_h @ V_h) @ Wo_cols.T   [S, E]
  - partial avg     avg_p = (1/16) * sum_h attn_h                [S, S]
Host sums the 4 partials per batch and adds bo.

On-core dataflow (per q-group of 512 columns):
  scoresT [k,q] psum (float32r matmuls from QT/KT head slices)
  -> ACT exp -> fp16 sbuf -> DVE multiply by transposed mask
  -> PE attn @ [V_h | ones] (T layout, no transposes needed): outT psum
     row 64 = softmax denominators -> DVE reciprocal -> PE broadcast
  -> outT normalized at eviction (DVE), attn normalized in place (DVE)
  -> avg: PE transpose-back of attn tiles with identity*(1/16),
     accumulating head pairs in fp16 PSUM; pair sums added at eviction.
"""

import sys

sys.path.insert(0, "/opt/trn_rl_repo")

import numpy as np

B, S, E, H = 2, 2048, 1024, 16
D = E // H  # 64
G = 4  # heads per core
DG = G * D  # 256 dims per core
NCORES = 8
SCALE = float(np.sqrt(D))

_cached = {}


def _build_nc():
    import concourse.bass as bass  # noqa: F401
    import concourse.tile as tile
    from concourse import bacc, mybir

    dt = mybir.dt
    AF = mybir.ActivationFunctionType
    OP = mybir.AluOpType

    P = 128
    EC = E // P  # 8 e-chunks
    DGC = DG // P  # 2 dg-chunks
    KC = S // P  # 16 k-chunks
    NQG = 4  # q groups of 512
    QW = S // NQG  # 512
    SH = 512  # s-quarter width for projection streaming
    SLOT = D + 1  # 65
    VW = G * SLOT  # 260

    nc = bacc.Bacc(None, target_bir_lowering=False)

    # ---- DRAM I/O ----
    xq_d = nc.dram_tensor("xq_t", [E, S], dt.float32r, kind="ExternalInput")
    xk_d = nc.dram_tensor("xk_t", [E, S], dt.float32r, kind="ExternalInput")
    xv_d = nc.dram_tensor("xv_t", [E, S], dt.float32r, kind="ExternalInput")
    mask_d = nc.dram_tensor("mask_t", [S, S], dt.float16, kind="ExternalInput")
    wq_d = nc.dram_tensor("wq_t", [E, DG], dt.float32r, kind="ExternalInput")
    wk_d = nc.dram_tensor("wk_t", [E, DG], dt.float32r, kind="ExternalInput")
    wv_d = nc.dram_tensor("wv_t", [E, VW], dt.float32r, kind="ExternalInput")
    wo_d = nc.dram_tensor("wo_t", [DG, E], dt.float16, kind="ExternalInput")
    bqs_d = nc.dram_tensor("bqs", [P, DGC], dt.float32, kind="ExternalInput")
    bks_d = nc.dram_tensor("bks", [P, DGC], dt.float32, kind="ExternalInput")
    bvs_d = nc.dram_tensor("bvs", [1, VW], dt.float32r, kind="ExternalInput")
    onesr_d = nc.dram_tensor("ones_r", [1, P], dt.float32r, kind="ExternalInput")
    id16_d = nc.dram_tensor("id16", [P, P], dt.float16, kind="ExternalInput")
    out_d = nc.dram_tensor("out_p", [S, E], dt.float32, kind="ExternalOutput")
    avg_d = nc.dram_tensor("avg_p", [S, S], dt.float16, kind="ExternalOutput")

    with tile.TileContext(nc) as tc:
        with (
            tc.tile_pool(name="consts", bufs=1) as consts,
            tc.tile_pool(name="xpool", bufs=8) as xpool,
            tc.tile_pool(name="wpool", bufs=2) as wpool,
            tc.tile_pool(name="qkv", bufs=1) as qkv,
            tc.tile_pool(name="maskp", bufs=3) as maskp,
            tc.tile_pool(name="expp", bufs=4) as expp,
            tc.tile_pool(name="small", bufs=2) as small,
            tc.tile_pool(name="avghalf", bufs=4) as avghalf,
            tc.tile_pool(name="stage", bufs=2) as stage,
            tc.tile_pool(name="ps_sc", bufs=2, space="PSUM") as ps_sc,
            tc.tile_pool(name="ps_ot", bufs=2, space="PSUM") as ps_ot,
            tc.tile_pool(name="ps_av", bufs=1, space="PSUM") as ps_av,
        ):
            # ---- constants ----
            onesr_s = consts.tile([1, P], dt.float32r)
            nc.sync.dma_start(out=onesr_s[:], in_=onesr_d[:])
            id16_s = consts.tile([P, P], dt.float16)
            nc.sync.dma_start(out=id16_s[:], in_=id16_d[:])
            bqs_s = consts.tile([P, DGC], dt.float32)
            nc.sync.dma_start(out=bqs_s[:], in_=bqs_d[:])
            bks_s = consts.tile([P, DGC], dt.float32)
            nc.sync.dma_start(out=bks_s[:], in_=bks_d[:])
            bvs_s = consts.tile([1, VW], dt.float32r)
            nc.sync.dma_start(out=bvs_s[:], in_=bvs_d[:])

            qt_s = qkv.tile([P, DGC, S], dt.float32r, tag="qt")
            kt_s = qkv.tile([P, DGC, S], dt.float32r, tag="kt")
            v_s = qkv.tile([P, KC, VW], dt.float16, tag="v")
            outn_s = qkv.tile([P, DGC, S], dt.float16, tag="outn")

            # ---- Q/K projections: psum[dg,qw] = sum_ec wT_ec.T @ x_ec ----
            for x_d, wname, wdram, b_s, out_s, scal in (
                (xk_d, "wk", wk_d, bks_s, kt_s, 1.0),
                (xq_d, "wq", wq_d, bqs_s, qt_s, 1.0 / SCALE),
            ):
                w_s = wpool.tile([P, EC, DG], dt.float32r, tag="w")
                nc.scalar.dma_start(
                    out=w_s[:], in_=wdram[:].rearrange("(ec p) d -> p ec d", p=P)
                )
                for sh in range(S // SH):
                    xch = []
                    for ec in range(EC):
                        xc = xpool.tile([P, SH], dt.float32r, tag="xc")
                        dma_eng = nc.sync if ec % 2 == 0 else nc.gpsimd
                        dma_eng.dma_start(
                            out=xc[:],
                            in_=x_d[ec * P : (ec + 1) * P, sh * SH : (sh + 1) * SH],
                        )
                        xch.append(xc)
                    for dgc in range(DGC):
                        for sb in range(SH // QW):
                            pp = ps_sc.tile([P, QW], dt.float32, tag="sc")
                            for ec in range(EC):
                                nc.tensor.matmul(
                                    pp[:],
                                    w_s[:, ec, dgc * P : (dgc + 1) * P],
                                    xch[ec][:, sb * QW : (sb + 1) * QW],
                                    start=(ec == 0),
                                    stop=(ec == EC - 1),
                                )
                            nc.vector.tensor_scalar(
                                out=out_s[
                                    :,
                                    dgc,
                                    sh * SH + sb * QW : sh * SH + (sb + 1) * QW,
                                ],
                                in0=pp[:],
                                scalar1=scal,
                                scalar2=b_s[:, dgc : dgc + 1],
                                op0=OP.mult,
                                op1=OP.add,
                            )
            # ---- V projection: psum[s,VW] = bias + sum_ec x_ec.T @ wvT_ec ----
            wv_s = wpool.tile([P, EC, VW], dt.float32r, tag="w")
            nc.scalar.dma_start(
                out=wv_s[:], in_=wv_d[:].rearrange("(ec p) d -> p ec d", p=P)
            )
            for sh in range(S // SH):
                xch = []
                for ec in range(EC):
                    xc = xpool.tile([P, SH], dt.float32r, tag="xc")
                    dma_eng = nc.sync if ec % 2 == 0 else nc.gpsimd
                    dma_eng.dma_start(
                        out=xc[:],
                        in_=xv_d[ec * P : (ec + 1) * P, sh * SH : (sh + 1) * SH],
                    )
                    xch.append(xc)
                for sb in range(SH // P):
                    pv = ps_sc.tile([P, VW], dt.float32, tag="sc")
                    nc.tensor.matmul(
                        pv[:], onesr_s[:], bvs_s[:], start=True, stop=False
                    )
                    for ec in range(EC):
                        nc.tensor.matmul(
                            pv[:],
                            xch[ec][:, sb * P : (sb + 1) * P],
                            wv_s[:, ec, :],
                            start=False,
                            stop=(ec == EC - 1),
                        )
                    nc.vector.tensor_copy(v_s[:, sh * (SH // P) + sb, :], pv[:])

            # ---- attention ----
            wo_s = wpool.tile([P, DGC, E], dt.float16, tag="w")
            nc.scalar.dma_start(
                out=wo_s[:], in_=wo_d[:].rearrange("(dc p) e -> p dc e", p=P)
            )
            ah_list = [None] * (QW // P)
            for qg in range(NQG):
                mk_halves = []
                for mh in range(2):
                    mk_s = maskp.tile([P, KC, QW // 2], dt.float16, tag="mask")
                    q0 = qg * QW + mh * (QW // 2)
                    nc.gpsimd.dma_start(
                        out=mk_s[:],
                        in_=mask_d[:].rearrange("(kc p) q -> p kc q", p=P)[
                            :, :, q0 : q0 + QW // 2
                        ],
                    )
                    mk_halves.append(mk_s)
                for pair in range(2):
                    pair_exp = []
                    for hh in range(2):
                        h = pair * 2 + hh
                        dgc = h // 2
                        prow = (h % 2) * D
                        ex_s = expp.tile([P, KC, QW], dt.float16, tag="expt")
                        pair_exp.append(ex_s)
                        # scoresT [k, q] in kc pairs; exp psum->sbuf fp16
                        for kp in range(KC // 2):
                            sc = ps_sc.tile([P, 2 * QW], dt.float32, tag="sc")
                            for j in range(2):
                                kc = 2 * kp + j
                                nc.tensor.matmul(
                                    sc[:, j * QW : (j + 1) * QW],
                                    kt_s[
                                        prow : prow + D, dgc, kc * P : (kc + 1) * P
                                    ],
                                    qt_s[
                                        prow : prow + D,
                                        dgc,
                                        qg * QW : (qg + 1) * QW,
                                    ],
                                    start=True,
                                    stop=True,
                                )
                            nc.scalar.activation(
                                ex_s[:, 2 * kp : 2 * kp + 2, :].rearrange(
                                    "p a b -> p (a b)"
                                ),
                                sc[:],
                                AF.Exp,
                            )
                        # mask multiply (in place, fp16, TT 2x mode),
                        # split by kc quarters so AV can start early
                        for kq in range(4):
                            kcs = slice(kq * (KC // 4), (kq + 1) * (KC // 4))
                            eng = nc.gpsimd if kq in (1, 2) else nc.vector
                            for mh in range(2):
                                qs = mh * (QW // 2)
                                eng.tensor_mul(
                                    ex_s[:, kcs, qs : qs + QW // 2],
                                    ex_s[:, kcs, qs : qs + QW // 2],
                                    mk_halves[mh][:, kcs, :],
                                )
                        # attn @ [V|1] : outT psum [65, qw]
                        ot = ps_ot.tile([SLOT, QW], dt.float32, tag="ot")
                        for kc in range(KC):
                            nc.tensor.matmul(
                                ot[:],
                                v_s[:, kc, h * SLOT : (h + 1) * SLOT],
                                ex_s[:, kc, :],
                                start=(kc == 0),
                                stop=(kc == KC - 1),
                            )
                        # reciprocal of sums; broadcast to partitions (POOL)
                        rrow = small.tile([1, QW], dt.float32, tag="rrow")
                        nc.vector.reciprocal(rrow[:], ot[D : D + 1, :])
                        rrow16 = small.tile([1, QW], dt.float16, tag="rrow16")
                        nc.scalar.copy(rrow16[:], rrow[:])
                        bc16 = small.tile([P, QW], dt.float16, tag="bc16")
                        nc.gpsimd.partition_broadcast(bc16[:], rrow16[:])
                        bc32 = small.tile([D, QW], dt.float32, tag="bc32")
                        nc.gpsimd.partition_broadcast(bc32[:], rrow[:], channels=D)
                        # normalized head output -> outn (fp16)
                        nc.vector.tensor_mul(
                            outn_s[
                                prow : prow + D, dgc, qg * QW : (qg + 1) * QW
                            ],
                            ot[:D, :],
                            bc32[:],
                        )
                        # normalize attn in place for the avg path (TT 2x),
                        # split by sq so avg matmuls can start early
                        for sq4 in range(4):
                            qs = sq4 * P
                            neng = nc.gpsimd if sq4 == 3 else nc.vector
                            neng.tensor_mul(
                                ex_s[:, :, qs : qs + P],
                                ex_s[:, :, qs : qs + P],
                                bc16[:, qs : qs + P]
                                .unsqueeze(1)
                                .broadcast_to([P, KC, P]),
                            )
                    # avg: transpose-back this head pair via regular fp16
                    # matmuls against the identity (transpose-mode is a pure
                    # permutation: rhs values ignored, no accumulation).
                    # fp32 psum accumulator, k-halves; 1/16 applied at evict.
                    for sq in range(QW // P):
                        avs = None
                        if pair == 1:
                            avs = stage.tile([P, S], dt.float16, tag="stage")
                        for kh in range(2):
                            av = ps_av.tile([P, S // 2], dt.float32, tag="av")
                            for kk in range(KC // 2):
                                kc = kh * (KC // 2) + kk
                                for hh in range(2):
                                    nc.tensor.matmul(
                                        av[:, kk * P : (kk + 1) * P],
                                        pair_exp[hh][:, kc, sq * P : (sq + 1) * P],
                                        id16_s[:],
                                        start=(hh == 0),
                                        stop=(hh == 1),
                                    )
                            ks = slice(kh * (S // 2), (kh + 1) * (S // 2))
                            if pair == 0:
                                if kh == 0:
                                    ah_list[sq] = avghalf.tile(
                                        [P, S], dt.float16, tag="ah", name="ah"
                                    )
                                    nc.scalar.copy(ah_list[sq][:, ks], av[:])
                                else:
                                    nc.vector.tensor_copy(
                                        ah_list[sq][:, ks], av[:]
                                    )
                            else:
                                nc.vector.tensor_add(
                                    avs[:, ks], av[:], ah_list[sq][:, ks]
                                )
                        if pair == 1:
                            r0 = qg * QW + sq * P
                            nc.gpsimd.dma_start(
                                out=avg_d[r0 : r0 + P, :], in_=avs[:]
                            )
                # ---- output projection for this q-group ----
                for sb4 in range(QW // P):
                    sb = qg * (QW // P) + sb4
                    po = ps_sc.tile([P, E], dt.float32, tag="sc")
                    for eb in range(2):
                        for dgc in range(DGC):
                            nc.tensor.matmul(
                                po[:, eb * QW : (eb + 1) * QW],
                                outn_s[:, dgc, sb * P : (sb + 1) * P],
                                wo_s[:, dgc, eb * QW : (eb + 1) * QW],
                                start=(dgc == 0),
                                stop=(dgc == DGC - 1),
                            )
                    os_ = stage.tile([P, E], dt.float32, tag="stage")
                    if sb4 % 2 == 0:
                        nc.scalar.copy(os_[:], po[:])
                    else:
                        nc.vector.tensor_copy(os_[:], po[:])
                    nc.sync.dma_start(
                        out=out_d[sb * P : (sb + 1) * P, :], in_=os_[:]
                    )


    nc.finalize()
    return nc


def _prep_core_inputs(inputs, b, g):
    f32 = np.float32
    q = np.ascontiguousarray(inputs["query"][b].T).astype(f32, copy=False)
    k = np.ascontiguousarray(inputs["key"][b].T).astype(f32, copy=False)
    v = np.ascontiguousarray(inputs["value"][b].T).astype(f32, copy=False)
    mask_t = np.ascontiguousarray(inputs["mask"][b].T).astype(np.float16)
    r0, r1 = g * DG, (g + 1) * DG
    wq_t = np.ascontiguousarray(inputs["Wq"][r0:r1, :].T).astype(f32, copy=False)
    wk_t = np.ascontiguousarray(inputs["Wk"][r0:r1, :].T).astype(f32, copy=False)
    wv = inputs["Wv"][r0:r1, :]  # [DG, E]
    SLOT = D + 1
    wv_t = np.zeros((E, G * SLOT), dtype=f32)
    bvs = np.zeros((1, G * SLOT), dtype=f32)
    bv = inputs["bv"][r0:r1]
    for h in range(G):
        wv_t[:, h * SLOT : h * SLOT + D] = wv[h * D : (h + 1) * D, :].T
        bvs[0, h * SLOT : h * SLOT + D] = bv[h * D : (h + 1) * D]
        bvs[0, h * SLOT + D] = 1.0
    wo_t = np.ascontiguousarray(inputs["Wo"][:, r0:r1].T).astype(np.float16)
    bqs = np.ascontiguousarray(inputs["bq"][r0:r1].reshape(2, 128).T).astype(f32)
    bks = np.ascontiguousarray(inputs["bk"][r0:r1].reshape(2, 128).T).astype(f32)
    id16 = (np.eye(128, dtype=np.float32) / 16.0).astype(np.float16)
    return {
        "xq_t": q,
        "xk_t": k,
        "xv_t": v,
        "mask_t": mask_t,
        "wq_t": wq_t,
        "wk_t": wk_t,
        "wv_t": wv_t,
        "wo_t": wo_t,
        "bqs": bqs,
        "bks": bks,
        "bvs": bvs,
        "ones_r": np.ones((1, 128), dtype=f32),
        "id16": id16,
    }


def kernel(query, key, value, mask, Wq, bq, Wk, bk, Wv, bv, Wo, bo, trace=False):
    from concourse.bass_utils import run_bass_kernel_spmd

    inputs = {
        "query": np.asarray(query),
        "key": np.asarray(key),
        "value": np.asarray(value),
        "mask": np.asarray(mask),
        "Wq": np.asarray(Wq),
        "bq": np.asarray(bq),
        "Wk": np.asarray(Wk),
        "bk": np.asarray(bk),
        "Wv": np.asarray(Wv),
        "bv": np.asarray(bv),
        "Wo": np.asarray(Wo),
        "bo": np.asarray(bo),
    }
    if "nc" not in _cached:
        _cached["nc"] = _build_nc()
    nc = _cached["nc"]

    in_maps = [_prep_core_inputs(inputs, c // G, c % G) for c in range(NCORES)]

    res = run_bass_kernel_spmd(nc, in_maps, core_ids=list(range(NCORES)), trace=trace)
    _cached["last_result"] = res

    output = np.zeros((B, S, E), dtype=np.float32)
    avg = np.zeros((B, S, S), dtype=np.float32)
    for c in range(NCORES):
        b = c // G
        output[b] += res.results[c]["out_p"]
        avg[b] += res.results[c]["avg_p"].astype(np.float32)
    output += inputs["bo"][None, None, :]
    return output, avg


if __name__ == "__main__":
    rng = np.random.default_rng(0)
    inp = {
        "query": rng.standard_normal((B, S, E), dtype=np.float32),
        "key": rng.standard_normal((B, S, E), dtype=np.float32),
        "value": rng.standard_normal((B, S, E), dtype=np.float32),
        "mask": rng.integers(0, 2, (B, S, S)).astype(np.int32),
        "Wq": (rng.standard_normal((E, E), dtype=np.float32) / np.sqrt(E)),
        "bq": np.zeros(E, np.float32),
        "Wk": (rng.standard_normal((E, E), dtype=np.float32) / np.sqrt(E)),
        "bk": np.zeros(E, np.float32),
        "Wv": (rng.standard_normal((E, E), dtype=np.float32) / np.sqrt(E)),
        "bv": np.zeros(E, np.float32),
        "Wo": (rng.standard_normal((E, E), dtype=np.float32) / np.sqrt(E)),
        "bo": np.zeros(E, np.float32),
    }
    out, avg = kernel(**inp)
    print("out", out.shape, "avg", avg.shape)
